# revision 8
# baseline (speedup 1.0000x reference)
"""CharRNN Trainium2 kernel.

Math (per batch row b):
    x_proj = emb_table[x] @ W_e            # == (emb_table @ W_e)[x]  (gather commutes)
    h_t    = tanh(x_proj[t] + h_{t-1} @ W_h)
    logits = outs @ W_o

Strategy: data-parallel over batch across 8 cores (32 rows each). On each
core the hidden state is kept TRANSPOSED in SBUF (H on partitions, batch on
free dim, as 8 chunks of [128, 32] living in a ring buffer) so the
recurrence matmul needs no per-step transpose of its stationary operand:

    z[b, n] = sum_k ht[k].T @ W_h[k, n]    lhsT = ht chunk [128, 32] (cheap load)
                                           rhs  = W_h chunk [128, 512] (streams)

float32r runs those at 1 cycle/row (full rate, moving dim >= 256). The
input projection is precomputed once as embW = emb_table @ W_e [256, 1024]
and per-step x_proj rows are fetched with an indirect-DMA gather (rows are
4KB contiguous). tanh output [32, 1024] is transposed back to ht layout
with 8 PE transposes per step. Output projection runs every 16 steps as a
batched matmul over the ring (N=512).
"""

from contextlib import ExitStack

import numpy as np
import concourse.bass as bass
import concourse.tile as tile
from concourse import bacc, mybir
from concourse.bass_utils import run_bass_kernel_spmd
from concourse.vector_clock import ScopedClock
from concourse.masks import make_identity

P = 128
B, L, V, E, H = 256, 512, 256, 256, 1024
NCORES = 8
BL = B // NCORES          # 32 batch rows per core
KC = H // P               # 8 contraction chunks
F32 = mybir.dt.float32
F32R = mybir.dt.float32r
I32 = mybir.dt.int32
TANH = mybir.ActivationFunctionType.Tanh


class _TC(tile.TileContext):
    """Walrus in this build lowers InstDrain with at most ONE sync wait
    (NEURON_ISA_TPB_CTRL_NO_STRUCT). Split the exit drain's global-clock
    waits across a chain of single-wait drains."""

    def _drain_and_barrier(self, tick_clock, wait_clock):
        nc = self.nc
        drain_inst = nc.sync.drain()
        wait_clock.add_sem_waits(
            drain_inst.ins, ScopedClock({None: tick_clock.global_clock})
        )
        si = drain_inst.ins.sync_info
        if si is not None and len(si.on_wait) > 1:
            waits = list(si.on_wait)
            upd = list(si.on_update)
            drain_inst.ins.sync_info = mybir.SyncInfo(on_wait=waits[:1], on_update=upd)
            for i in range(1, len(waits)):
                d2 = nc.sync.drain()
                d2.ins.sync_info = mybir.SyncInfo(on_wait=[waits[i]], on_update=[])
        nc.all_engine_barrier()
        popped = nc._tile_sem_poison_stack.pop()
        assert popped is self._sem_poison
        nc.clear_and_free_semaphores(list(self.sems.allocated().values()))
        nc.all_engine_barrier()


def _r(ap):
    return ap.bitcast(F32R)


def build(L_steps=L, blk=16):
    """Build the per-core Bass program (SPMD: all cores run this)."""
    assert L_steps % blk == 0
    slots = 2 * blk

    nc = bacc.Bacc("TRN2", target_bir_lowering=False, debug=False, num_devices=NCORES)
    x_d = nc.dram_tensor("x", [BL, L_steps], I32, kind="ExternalInput").ap()
    h0_d = nc.dram_tensor("hidden", [BL, H], F32, kind="ExternalInput").ap()
    et_d = nc.dram_tensor("emb_table", [V, E], F32, kind="ExternalInput").ap()
    we_d = nc.dram_tensor("W_e", [E, H], F32, kind="ExternalInput").ap()
    wh_d = nc.dram_tensor("W_h", [H, H], F32, kind="ExternalInput").ap()
    wo_d = nc.dram_tensor("W_o", [H, V], F32, kind="ExternalInput").ap()
    lg_d = nc.dram_tensor("logits", [BL, L_steps, V], F32, kind="ExternalOutput").ap()
    hf_d = nc.dram_tensor("hfinal", [BL, H], F32, kind="ExternalOutput").ap()

    with _TC(nc) as tc, ExitStack() as ctx:
        pers = ctx.enter_context(tc.tile_pool(name="pers", bufs=1))
        dram = ctx.enter_context(tc.tile_pool(name="dram", bufs=1, space="DRAM"))

        # --- persistent SBUF tensors (matmul operands are F32R: walrus
        # requires fp32r-consumed data to be produced by a rounding op, so
        # DMA into f32 staging then engine-copy with dtype conversion)
        wh_sb = pers.tile([P, KC * H], F32R)  # W_h chunk k at cols [H*k, H*k+H)
        wo_sb = pers.tile([P, KC * V], F32R)  # W_o chunk k at cols [V*k, ...)
        we_sb = pers.tile([P, 2 * H], F32R)  # W_e chunk e at cols [H*e, ...)
        etab_sb = pers.tile([P, 2 * E], F32)  # emb_table chunk c at cols [E*c, ...)
        nc.sync.dma_start(etab_sb[:].rearrange("p (c e) -> p c e", c=2), et_d.rearrange("(c p) e -> p c e", p=P))
        x_sb = pers.tile([BL, L_steps], I32)
        nc.sync.dma_start(x_sb[:], x_d[:])
        ident = pers.tile([P, P], F32)
        make_identity(nc, ident[:])
        # ring of transposed hidden states: chunk k, slot s at
        # cols [k*slots*BL + s*BL, ... + BL)
        ring = pers.tile([P, KC * slots * BL], F32R)
        embW_dram = dram.tile([V, H], F32)

        # --- startup: embW = emb_table @ W_e  (DRAM, rows contiguous for gather)
        with tc.tile_pool(name="init_ps", bufs=2, space="PSUM") as ips, tc.tile_pool(
            name="init_sb", bufs=2
        ) as isb:
            stage = isb.tile([P, KC * H], F32, tag="stage", bufs=1)
            nc.sync.dma_start(stage[:].rearrange("p (k n) -> p k n", k=KC), wh_d.rearrange("(k p) n -> p k n", p=P))
            nc.vector.tensor_copy(wh_sb[:], stage[:])
            stage = isb.tile([P, KC * H], F32, tag="stage", bufs=1, name="stage2")
            nc.sync.dma_start(stage[:, : KC * V].rearrange("p (k v) -> p k v", k=KC), wo_d.rearrange("(k p) v -> p k v", p=P))
            nc.scalar.copy(wo_sb[:], stage[:, : KC * V])
            stage = isb.tile([P, KC * H], F32, tag="stage", bufs=1, name="stage3")
            nc.sync.dma_start(stage[:, : 2 * H].rearrange("p (e h) -> p e h", e=2), we_d.rearrange("(e p) h -> p e h", p=P))
            nc.scalar.copy(we_sb[:], stage[:, : 2 * H])
            etT = pers.tile([P, 2 * V], F32R)  # e-chunk ec at cols [V*ec, ...)
            for c in range(2):  # v chunk of emb_table rows
                for ec in range(2):  # e chunk
                    ptr0 = ips.tile([P, P], F32, tag="ptr0")
                    nc.tensor.transpose(
                        ptr0[:], etab_sb[:, c * E + ec * P : c * E + (ec + 1) * P], ident[:]
                    )
                    nc.vector.tensor_copy(
                        etT[:, ec * V + c * P : ec * V + (c + 1) * P], ptr0[:]
                    )
            for vc in range(2):  # embW row chunk
                for nh in range(2):  # embW col half
                    pe_ps = ips.tile([P, 512], F32, tag="pe_ps")
                    for ec in range(2):
                        nc.tensor.matmul(
                            pe_ps[:],
                            lhsT=etT[:, ec * V + vc * P : ec * V + (vc + 1) * P],
                            rhs=we_sb[:, ec * H + nh * 512 : ec * H + (nh + 1) * 512],
                            start=(ec == 0),
                            stop=(ec == 1),
                        )
                    ew_sb = isb.tile([P, 512], F32, tag="ew_sb")
                    nc.vector.tensor_copy(ew_sb[:], pe_ps[:])
                    nc.sync.dma_start(
                        embW_dram[vc * P : (vc + 1) * P, nh * 512 : (nh + 1) * 512],
                        ew_sb[:],
                    )

        # --- main pools
        gat = ctx.enter_context(tc.tile_pool(name="gat", bufs=6))
        work = ctx.enter_context(tc.tile_pool(name="work", bufs=4))
        psz = ctx.enter_context(tc.tile_pool(name="psz", bufs=4, space="PSUM"))
        pst = ctx.enter_context(tc.tile_pool(name="pst", bufs=2, space="PSUM"))
        psp = ctx.enter_context(tc.tile_pool(name="psp", bufs=2, space="PSUM"))

        def transpose_to_ring(h_ap, slot):
            """h [32, 1024] (batch on partitions) -> ring chunks [128, 32] at slot."""
            ptr = pst.tile([P, KC * BL], F32, tag="ptr")
            for j in range(KC):
                nc.tensor.transpose(
                    ptr[:, j * BL : (j + 1) * BL],
                    h_ap[:, j * P : (j + 1) * P],
                    ident[:BL, :BL],
                )
            # one strided copy: ptr [128, (8, 32)] -> ring [128, (8 @ stride slots*BL, 32)]
            dst = ring[:].rearrange("p (k sb) -> p k sb", k=KC)[
                :, :, slot * BL : slot * BL + BL
            ]
            nc.vector.tensor_copy(dst, ptr[:].rearrange("p (k b) -> p k b", k=KC))

        # initial hidden -> ring slot (slots-1)
        h0_sb = work.tile([BL, H], F32, tag="h_sb")
        nc.sync.dma_start(h0_sb[:], h0_d[:])
        transpose_to_ring(h0_sb, slots - 1)

        prev_slot = slots - 1
        h_sb = None
        for t in range(L_steps):
            slot = t % slots
            # gather x_proj rows for this step
            xq = gat.tile([BL, H], F32, tag="xq")
            nc.gpsimd.indirect_dma_start(
                out=xq[:],
                out_offset=None,
                in_=embW_dram[:, :],
                in_offset=bass.IndirectOffsetOnAxis(ap=x_sb[:, t : t + 1], axis=0),
            )
            # z = ht_{t-1}.T @ W_h   (accumulated over KC chunks, 2 halves of 512)
            pz = []
            for nh in range(2):
                ps = psz.tile([BL, 512], F32, tag="psz")
                base = prev_slot * BL
                for k in range(KC):
                    nc.tensor.matmul(
                        ps[:],
                        lhsT=ring[:, k * slots * BL + base : k * slots * BL + base + BL],
                        rhs=wh_sb[:, k * H + nh * 512 : k * H + nh * 512 + 512],
                        start=(k == 0),
                        stop=(k == KC - 1),
                    )
                pz.append(ps)
            # h = tanh(z + x_proj)
            h_sb = work.tile([BL, H], F32, tag="h_sb")
            for nh in range(2):
                zt = work.tile([BL, 512], F32, tag="zt")
                nc.vector.tensor_add(
                    zt[:], pz[nh][:], xq[:, nh * 512 : (nh + 1) * 512]
                )
                nc.scalar.activation(h_sb[:, nh * 512 : (nh + 1) * 512], zt[:], TANH)
            transpose_to_ring(h_sb, slot)
            prev_slot = slot

            # output projection over the last `blk` steps
            if t % blk == blk - 1:
                bi = t // blk
                par = (bi % 2) * blk * BL
                pps = []
                for mv in range(2):
                    pp = psp.tile([P, blk * BL], F32, tag="pp")
                    for k in range(KC):
                        nc.tensor.matmul(
                            pp[:],
                            lhsT=wo_sb[:, k * V + mv * P : k * V + (mv + 1) * P],
                            rhs=ring[:, k * slots * BL + par : k * slots * BL + par + blk * BL],
                            start=(k == 0),
                            stop=(k == KC - 1),
                        )
                    lg_sb = work.tile([P, blk * BL], F32, tag="lg", name=f"lg{mv}")
                    nc.vector.tensor_copy(lg_sb[:], pp[:])
                    pps.append(lg_sb)
                # transpose [v, b] tiles to [b, v] and assemble the block
                lgb = work.tile([BL, blk * V], F32, tag="lgb", bufs=2)
                for s in range(blk):
                    lgt = pst.tile([BL, V], F32, tag="ptr", name="lgt")
                    for mv in range(2):
                        nc.tensor.transpose(
                            lgt[:, mv * P : (mv + 1) * P],
                            pps[mv][:, s * BL : (s + 1) * BL],
                            ident[:],
                        )
                    nc.vector.tensor_copy(lgb[:, s * V : (s + 1) * V], lgt[:])
                nc.sync.dma_start(
                    lg_d[:, bi * blk : (bi + 1) * blk, :].rearrange("b s v -> b (s v)"),
                    lgb[:],
                )

        nc.sync.dma_start(hf_d[:], h_sb[:])
    nc.compile()
    return nc


_CACHE = {}


def _get_nc():
    if "nc" not in _CACHE:
        _CACHE["nc"] = build()
    return _CACHE["nc"]


def kernel(x, hidden, emb_table, W_e, W_h, W_o):
    x = np.ascontiguousarray(np.asarray(x).astype(np.int32))
    hidden = np.ascontiguousarray(np.asarray(hidden, dtype=np.float32))
    emb_table = np.ascontiguousarray(np.asarray(emb_table, dtype=np.float32))
    W_e = np.ascontiguousarray(np.asarray(W_e, dtype=np.float32))
    W_h = np.ascontiguousarray(np.asarray(W_h, dtype=np.float32))
    W_o = np.ascontiguousarray(np.asarray(W_o, dtype=np.float32))

    nc = _get_nc()
    in_maps = []
    for c in range(NCORES):
        sl = slice(c * BL, (c + 1) * BL)
        in_maps.append(
            {
                "x": x[sl],
                "hidden": hidden[sl],
                "emb_table": emb_table,
                "W_e": W_e,
                "W_h": W_h,
                "W_o": W_o,
            }
        )
    res = run_bass_kernel_spmd(nc, in_maps, core_ids=list(range(NCORES)), trace=False)
    logits = np.concatenate([res.results[c]["logits"] for c in range(NCORES)], axis=0)
    hfinal = np.concatenate([res.results[c]["hfinal"] for c in range(NCORES)], axis=0)
    return logits, hfinal


# revision 12
# speedup vs baseline: 3.2732x; 3.2732x over previous
"""CharRNN Trainium2 kernel.

Math (per batch row b):
    x_proj = emb_table[x] @ W_e            # == (emb_table @ W_e)[x]  (gather commutes)
    h_t    = tanh(x_proj[t] + h_{t-1} @ W_h)
    logits = outs @ W_o

Strategy: data-parallel over batch across 8 cores (32 rows each). On each
core the hidden state is kept TRANSPOSED in SBUF (H on partitions, batch on
free dim, as 8 chunks of [128, 32] living in a ring buffer) so the
recurrence matmul needs no per-step transpose of its stationary operand:

    z[b, n] = sum_k ht[k].T @ W_h[k, n]    lhsT = ht chunk [128, 32] (cheap load)
                                           rhs  = W_h chunk [128, 512] (streams)

float32r runs those at 1 cycle/row (full rate, moving dim >= 256). The
input projection is precomputed once as embW = emb_table @ W_e [256, 1024]
and per-step x_proj rows are fetched with an indirect-DMA gather (rows are
4KB contiguous). tanh output [32, 1024] is transposed back to ht layout
with 8 PE transposes per step. Output projection runs every 16 steps as a
batched matmul over the ring (N=512).
"""

from contextlib import ExitStack

import numpy as np
import concourse.bass as bass
import concourse.tile as tile
from concourse import bacc, mybir
from concourse.bass_utils import run_bass_kernel_spmd
from concourse.vector_clock import ScopedClock
from concourse.masks import make_identity

P = 128
B, L, V, E, H = 256, 512, 256, 256, 1024
NCORES = 8
BL = B // NCORES          # 32 batch rows per core
KC = H // P               # 8 contraction chunks
F32 = mybir.dt.float32
F32R = mybir.dt.float32r
I32 = mybir.dt.int32
TANH = mybir.ActivationFunctionType.Tanh


class _TC(tile.TileContext):
    """Walrus in this build lowers InstDrain with at most ONE sync wait
    (NEURON_ISA_TPB_CTRL_NO_STRUCT). Split the exit drain's global-clock
    waits across a chain of single-wait drains."""

    def _drain_and_barrier(self, tick_clock, wait_clock):
        nc = self.nc
        drain_inst = nc.sync.drain()
        wait_clock.add_sem_waits(
            drain_inst.ins, ScopedClock({None: tick_clock.global_clock})
        )
        si = drain_inst.ins.sync_info
        if si is not None and len(si.on_wait) > 1:
            waits = list(si.on_wait)
            upd = list(si.on_update)
            drain_inst.ins.sync_info = mybir.SyncInfo(on_wait=waits[:1], on_update=upd)
            for i in range(1, len(waits)):
                d2 = nc.sync.drain()
                d2.ins.sync_info = mybir.SyncInfo(on_wait=[waits[i]], on_update=[])
        nc.all_engine_barrier()
        popped = nc._tile_sem_poison_stack.pop()
        assert popped is self._sem_poison
        nc.clear_and_free_semaphores(list(self.sems.allocated().values()))
        nc.all_engine_barrier()


def _r(ap):
    return ap.bitcast(F32R)


def build(L_steps=L, blk=16, repeat=1):
    """Build the per-core Bass program (SPMD: all cores run this).
    repeat>1 reruns the main loop (timing experiments only)."""
    assert L_steps % blk == 0
    slots = 2 * blk

    nc = bacc.Bacc("TRN2", target_bir_lowering=False, debug=False, num_devices=NCORES)
    x_d = nc.dram_tensor("x", [BL, L_steps], I32, kind="ExternalInput").ap()
    h0_d = nc.dram_tensor("hidden", [BL, H], F32, kind="ExternalInput").ap()
    et_d = nc.dram_tensor("emb_table", [V, E], F32, kind="ExternalInput").ap()
    we_d = nc.dram_tensor("W_e", [E, H], F32, kind="ExternalInput").ap()
    wh_d = nc.dram_tensor("W_h", [H, H], F32, kind="ExternalInput").ap()
    wo_d = nc.dram_tensor("W_o", [H, V], F32, kind="ExternalInput").ap()
    lg_d = nc.dram_tensor("logits", [BL, L_steps, V], F32, kind="ExternalOutput").ap()
    hf_d = nc.dram_tensor("hfinal", [BL, H], F32, kind="ExternalOutput").ap()

    with _TC(nc) as tc, ExitStack() as ctx:
        pers = ctx.enter_context(tc.tile_pool(name="pers", bufs=1))
        dram = ctx.enter_context(tc.tile_pool(name="dram", bufs=1, space="DRAM"))

        # --- persistent SBUF tensors (matmul operands are F32R: walrus
        # requires fp32r-consumed data to be produced by a rounding op, so
        # DMA into f32 staging then engine-copy with dtype conversion)
        wh_sb = pers.tile([P, KC * H], F32R)  # W_h chunk k at cols [H*k, H*k+H)
        wo_sb = pers.tile([P, KC * V], F32R)  # W_o chunk k at cols [V*k, ...)
        we_sb = pers.tile([P, 2 * H], F32R)  # W_e chunk e at cols [H*e, ...)
        etab_sb = pers.tile([P, 2 * E], F32)  # emb_table chunk c at cols [E*c, ...)
        nc.sync.dma_start(etab_sb[:].rearrange("p (c e) -> p c e", c=2), et_d.rearrange("(c p) e -> p c e", p=P))
        x_sb = pers.tile([BL, L_steps], I32)
        nc.sync.dma_start(x_sb[:], x_d[:])
        ident = pers.tile([P, P], F32)
        make_identity(nc, ident[:])
        identr = pers.tile([BL, BL], F32R)
        nc.vector.tensor_copy(identr[:], ident[:BL, :BL])
        # ring of transposed hidden states: chunk k, slot s at
        # cols [k*slots*BL + s*BL, ... + BL)
        ring = pers.tile([P, KC * slots * BL], F32R)
        embW_dram = dram.tile([V, H], F32)

        # --- startup: embW = emb_table @ W_e  (DRAM, rows contiguous for gather)
        with tc.tile_pool(name="init_ps", bufs=2, space="PSUM") as ips, tc.tile_pool(
            name="init_sb", bufs=2
        ) as isb:
            stage = isb.tile([P, KC * H], F32, tag="stage", bufs=1)
            nc.sync.dma_start(stage[:].rearrange("p (k n) -> p k n", k=KC), wh_d.rearrange("(k p) n -> p k n", p=P))
            nc.vector.tensor_copy(wh_sb[:], stage[:])
            stage = isb.tile([P, KC * H], F32, tag="stage", bufs=1, name="stage2")
            nc.sync.dma_start(stage[:, : KC * V].rearrange("p (k v) -> p k v", k=KC), wo_d.rearrange("(k p) v -> p k v", p=P))
            nc.scalar.copy(wo_sb[:], stage[:, : KC * V])
            stage = isb.tile([P, KC * H], F32, tag="stage", bufs=1, name="stage3")
            nc.sync.dma_start(stage[:, : 2 * H].rearrange("p (e h) -> p e h", e=2), we_d.rearrange("(e p) h -> p e h", p=P))
            nc.scalar.copy(we_sb[:], stage[:, : 2 * H])
            etT = pers.tile([P, 2 * V], F32R)  # e-chunk ec at cols [V*ec, ...)
            for c in range(2):  # v chunk of emb_table rows
                for ec in range(2):  # e chunk
                    ptr0 = ips.tile([P, P], F32, tag="ptr0")
                    nc.tensor.transpose(
                        ptr0[:], etab_sb[:, c * E + ec * P : c * E + (ec + 1) * P], ident[:]
                    )
                    nc.vector.tensor_copy(
                        etT[:, ec * V + c * P : ec * V + (c + 1) * P], ptr0[:]
                    )
            for vc in range(2):  # embW row chunk
                for nh in range(2):  # embW col half
                    pe_ps = ips.tile([P, 512], F32, tag="pe_ps")
                    for ec in range(2):
                        nc.tensor.matmul(
                            pe_ps[:],
                            lhsT=etT[:, ec * V + vc * P : ec * V + (vc + 1) * P],
                            rhs=we_sb[:, ec * H + nh * 512 : ec * H + (nh + 1) * 512],
                            start=(ec == 0),
                            stop=(ec == 1),
                        )
                    ew_sb = isb.tile([P, 512], F32, tag="ew_sb")
                    nc.vector.tensor_copy(ew_sb[:], pe_ps[:])
                    nc.sync.dma_start(
                        embW_dram[vc * P : (vc + 1) * P, nh * 512 : (nh + 1) * 512],
                        ew_sb[:],
                    )

        # --- main pools
        gat = ctx.enter_context(tc.tile_pool(name="gat", bufs=6))
        work = ctx.enter_context(tc.tile_pool(name="work", bufs=4))
        psz = ctx.enter_context(tc.tile_pool(name="psz", bufs=4, space="PSUM"))
        pst = ctx.enter_context(tc.tile_pool(name="pst", bufs=2, space="PSUM"))
        psp = ctx.enter_context(tc.tile_pool(name="psp", bufs=2, space="PSUM"))

        def transpose_to_ring(h_ap, slot, nh):
            """h half [32, 512] (batch on partitions) -> ring chunks 4nh..4nh+3."""
            kh = KC // 2
            ptr = pst.tile([P, kh * BL], F32, tag="ptr", name=f"ptr{nh}")
            for j in range(kh):
                nc.tensor.transpose(
                    ptr[:, j * BL : (j + 1) * BL],
                    h_ap[:, j * P : (j + 1) * P],
                    ident[:BL, :BL],
                )
            # strided copy: ptr [128, (4, 32)] -> ring chunks (4nh+j) at slot
            dst = ring[:].rearrange("p (k sb) -> p k sb", k=KC)[
                :, nh * kh : (nh + 1) * kh, slot * BL : slot * BL + BL
            ]
            nc.vector.tensor_copy(dst, ptr[:].rearrange("p (k b) -> p k b", k=kh))

        # initial hidden -> ring slot (slots-1)
        h0_sb = work.tile([BL, H], F32, tag="h_sb")
        nc.sync.dma_start(h0_sb[:], h0_d[:])
        transpose_to_ring(h0_sb[:, :512], slots - 1, 0)
        transpose_to_ring(h0_sb[:, 512:], slots - 1, 1)

        prev_slot = slots - 1
        h_sb = None
        for t_rep in range(L_steps * repeat):
            t = t_rep % L_steps
            slot = t % slots
            # gather x_proj rows for this step
            xq = gat.tile([BL, H], F32, tag="xq")
            nc.gpsimd.indirect_dma_start(
                out=xq[:],
                out_offset=None,
                in_=embW_dram[:, :],
                in_offset=bass.IndirectOffsetOnAxis(ap=x_sb[:, t : t + 1], axis=0),
            )
            xqr = gat.tile([BL, H], F32R, tag="xqr")
            nc.vector.tensor_copy(xqr[:], xq[:])
            # z = ht_{t-1}.T @ W_h   (accumulated over KC chunks, 2 halves of 512)
            pz = []
            for nh in range(2):
                ps = psz.tile([BL, 512], F32, tag="psz")
                base = prev_slot * BL
                nc.tensor.matmul(
                    ps[:],
                    lhsT=identr[:],
                    rhs=xqr[:, nh * 512 : (nh + 1) * 512],
                    start=True,
                    stop=False,
                )
                for k in range(KC):
                    nc.tensor.matmul(
                        ps[:],
                        lhsT=ring[:, k * slots * BL + base : k * slots * BL + base + BL],
                        rhs=wh_sb[:, k * H + nh * 512 : k * H + nh * 512 + 512],
                        start=False,
                        stop=(k == KC - 1),
                    )
                pz.append(ps)
            # h = tanh(z + x_proj), then back to ring layout — per half so
            # chunks 0-3 are ready while half 1 is still in matmuls
            h_sb = work.tile([BL, H], F32, tag="h_sb")
            for nh in range(2):
                nc.scalar.activation(h_sb[:, nh * 512 : (nh + 1) * 512], pz[nh][:], TANH)
                transpose_to_ring(h_sb[:, nh * 512 : (nh + 1) * 512], slot, nh)
            prev_slot = slot

            # output projection over the last `blk` steps
            if t % blk == blk - 1:
                bi = t // blk
                par = (bi % 2) * blk * BL
                pps = []
                for mv in range(2):
                    pp = psp.tile([P, blk * BL], F32, tag="pp")
                    for k in range(KC):
                        nc.tensor.matmul(
                            pp[:],
                            lhsT=wo_sb[:, k * V + mv * P : k * V + (mv + 1) * P],
                            rhs=ring[:, k * slots * BL + par : k * slots * BL + par + blk * BL],
                            start=(k == 0),
                            stop=(k == KC - 1),
                        )
                    lg_sb = work.tile([P, blk * BL], F32, tag="lg", name=f"lg{mv}")
                    nc.vector.tensor_copy(lg_sb[:], pp[:])
                    pps.append(lg_sb)
                # transpose [v, b] tiles to [b, v] and assemble the block
                lgb = work.tile([BL, blk * V], F32, tag="lgb", bufs=2)
                for s in range(blk):
                    lgt = pst.tile([BL, V], F32, tag="ptr", name="lgt")
                    for mv in range(2):
                        nc.tensor.transpose(
                            lgt[:, mv * P : (mv + 1) * P],
                            pps[mv][:, s * BL : (s + 1) * BL],
                            ident[:],
                        )
                    nc.vector.tensor_copy(lgb[:, s * V : (s + 1) * V], lgt[:])
                nc.sync.dma_start(
                    lg_d[:, bi * blk : (bi + 1) * blk, :].rearrange("b s v -> b (s v)"),
                    lgb[:],
                )

        nc.sync.dma_start(hf_d[:], h_sb[:])
    nc.compile()
    return nc


_CACHE = {}


def _get_nc():
    if "nc" not in _CACHE:
        _CACHE["nc"] = build()
    return _CACHE["nc"]


def kernel(x, hidden, emb_table, W_e, W_h, W_o):
    x = np.ascontiguousarray(np.asarray(x).astype(np.int32))
    hidden = np.ascontiguousarray(np.asarray(hidden, dtype=np.float32))
    emb_table = np.ascontiguousarray(np.asarray(emb_table, dtype=np.float32))
    W_e = np.ascontiguousarray(np.asarray(W_e, dtype=np.float32))
    W_h = np.ascontiguousarray(np.asarray(W_h, dtype=np.float32))
    W_o = np.ascontiguousarray(np.asarray(W_o, dtype=np.float32))

    nc = _get_nc()
    in_maps = []
    for c in range(NCORES):
        sl = slice(c * BL, (c + 1) * BL)
        in_maps.append(
            {
                "x": x[sl],
                "hidden": hidden[sl],
                "emb_table": emb_table,
                "W_e": W_e,
                "W_h": W_h,
                "W_o": W_o,
            }
        )
    res = run_bass_kernel_spmd(nc, in_maps, core_ids=list(range(NCORES)), trace=False)
    logits = np.concatenate([res.results[c]["logits"] for c in range(NCORES)], axis=0)
    hfinal = np.concatenate([res.results[c]["hfinal"] for c in range(NCORES)], axis=0)
    return logits, hfinal


# revision 13
# speedup vs baseline: 16.7594x; 5.1203x over previous
"""CharRNN Trainium2 kernel.

Math (per batch row b):
    x_proj = emb_table[x] @ W_e            # == (emb_table @ W_e)[x]  (gather commutes)
    h_t    = tanh(x_proj[t] + h_{t-1} @ W_h)
    logits = outs @ W_o

Strategy: data-parallel over batch across 8 cores (32 rows each). On each
core the hidden state is kept TRANSPOSED in SBUF (H on partitions, batch on
free dim, as 8 chunks of [128, 32] living in a ring buffer) so the
recurrence matmul needs no per-step transpose of its stationary operand:

    z[b, n] = sum_k ht[k].T @ W_h[k, n]    lhsT = ht chunk [128, 32] (cheap load)
                                           rhs  = W_h chunk [128, 512] (streams)

float32r runs those at 1 cycle/row (full rate, moving dim >= 256). The
input projection is precomputed once as embW = emb_table @ W_e [256, 1024]
and per-step x_proj rows are fetched with an indirect-DMA gather (rows are
4KB contiguous). tanh output [32, 1024] is transposed back to ht layout
with 8 PE transposes per step. Output projection runs every 16 steps as a
batched matmul over the ring (N=512).
"""

from contextlib import ExitStack

import numpy as np
import concourse.bass as bass
import concourse.tile as tile
from concourse import bacc, mybir
from concourse.bass_utils import run_bass_kernel_spmd
from concourse.vector_clock import ScopedClock
from concourse.masks import make_identity

P = 128
B, L, V, E, H = 256, 512, 256, 256, 1024
NCORES = 8
BL = B // NCORES          # 32 batch rows per core
KC = H // P               # 8 contraction chunks
F32 = mybir.dt.float32
F32R = mybir.dt.float32r
I32 = mybir.dt.int32
TANH = mybir.ActivationFunctionType.Tanh


class _TC(tile.TileContext):
    """Walrus in this build lowers InstDrain with at most ONE sync wait
    (NEURON_ISA_TPB_CTRL_NO_STRUCT). Split the exit drain's global-clock
    waits across a chain of single-wait drains."""

    def _drain_and_barrier(self, tick_clock, wait_clock):
        nc = self.nc
        drain_inst = nc.sync.drain()
        wait_clock.add_sem_waits(
            drain_inst.ins, ScopedClock({None: tick_clock.global_clock})
        )
        si = drain_inst.ins.sync_info
        if si is not None and len(si.on_wait) > 1:
            waits = list(si.on_wait)
            upd = list(si.on_update)
            drain_inst.ins.sync_info = mybir.SyncInfo(on_wait=waits[:1], on_update=upd)
            for i in range(1, len(waits)):
                d2 = nc.sync.drain()
                d2.ins.sync_info = mybir.SyncInfo(on_wait=[waits[i]], on_update=[])
        nc.all_engine_barrier()
        popped = nc._tile_sem_poison_stack.pop()
        assert popped is self._sem_poison
        nc.clear_and_free_semaphores(list(self.sems.allocated().values()))
        nc.all_engine_barrier()


def _r(ap):
    return ap.bitcast(F32R)


def build(L_steps=L, blk=16, repeat=1, ablate_gather=False):
    """Build the per-core Bass program (SPMD: all cores run this).
    repeat>1 reruns the main loop (timing experiments only)."""
    assert L_steps % blk == 0
    slots = 2 * blk

    nc = bacc.Bacc("TRN2", target_bir_lowering=False, debug=False, num_devices=NCORES)
    x_d = nc.dram_tensor("x", [BL, L_steps], I32, kind="ExternalInput").ap()
    h0_d = nc.dram_tensor("hidden", [BL, H], F32, kind="ExternalInput").ap()
    et_d = nc.dram_tensor("emb_table", [V, E], F32, kind="ExternalInput").ap()
    we_d = nc.dram_tensor("W_e", [E, H], F32, kind="ExternalInput").ap()
    wh_d = nc.dram_tensor("W_h", [H, H], F32, kind="ExternalInput").ap()
    wo_d = nc.dram_tensor("W_o", [H, V], F32, kind="ExternalInput").ap()
    lg_d = nc.dram_tensor("logits", [BL, L_steps, V], F32, kind="ExternalOutput").ap()
    hf_d = nc.dram_tensor("hfinal", [BL, H], F32, kind="ExternalOutput").ap()

    with _TC(nc) as tc, ExitStack() as ctx:
        pers = ctx.enter_context(tc.tile_pool(name="pers", bufs=1))
        dram = ctx.enter_context(tc.tile_pool(name="dram", bufs=1, space="DRAM"))

        # --- persistent SBUF tensors (matmul operands are F32R: walrus
        # requires fp32r-consumed data to be produced by a rounding op, so
        # DMA into f32 staging then engine-copy with dtype conversion)
        wh_sb = pers.tile([P, KC * H], F32R)  # W_h chunk k at cols [H*k, H*k+H)
        wo_sb = pers.tile([P, KC * V], F32R)  # W_o chunk k at cols [V*k, ...)
        we_sb = pers.tile([P, 2 * H], F32R)  # W_e chunk e at cols [H*e, ...)
        etab_sb = pers.tile([P, 2 * E], F32)  # emb_table chunk c at cols [E*c, ...)
        nc.sync.dma_start(etab_sb[:].rearrange("p (c e) -> p c e", c=2), et_d.rearrange("(c p) e -> p c e", p=P))
        x_sb = pers.tile([BL, L_steps], I32)
        nc.sync.dma_start(x_sb[:], x_d[:])
        ident = pers.tile([P, P], F32)
        make_identity(nc, ident[:])
        identr = pers.tile([BL, BL], F32R)
        nc.vector.tensor_copy(identr[:], ident[:BL, :BL])
        # ring of transposed hidden states: chunk k, slot s at
        # cols [k*slots*BL + s*BL, ... + BL)
        ring = pers.tile([P, KC * slots * BL], F32R)
        embW_dram = dram.tile([V, H], F32)

        # --- startup: embW = emb_table @ W_e  (DRAM, rows contiguous for gather)
        with tc.tile_pool(name="init_ps", bufs=2, space="PSUM") as ips, tc.tile_pool(
            name="init_sb", bufs=2
        ) as isb:
            stage = isb.tile([P, KC * H], F32, tag="stage", bufs=1)
            nc.sync.dma_start(stage[:].rearrange("p (k n) -> p k n", k=KC), wh_d.rearrange("(k p) n -> p k n", p=P))
            nc.vector.tensor_copy(wh_sb[:], stage[:])
            stage = isb.tile([P, KC * H], F32, tag="stage", bufs=1, name="stage2")
            nc.sync.dma_start(stage[:, : KC * V].rearrange("p (k v) -> p k v", k=KC), wo_d.rearrange("(k p) v -> p k v", p=P))
            nc.scalar.copy(wo_sb[:], stage[:, : KC * V])
            stage = isb.tile([P, KC * H], F32, tag="stage", bufs=1, name="stage3")
            nc.sync.dma_start(stage[:, : 2 * H].rearrange("p (e h) -> p e h", e=2), we_d.rearrange("(e p) h -> p e h", p=P))
            nc.scalar.copy(we_sb[:], stage[:, : 2 * H])
            etT = pers.tile([P, 2 * V], F32R)  # e-chunk ec at cols [V*ec, ...)
            for c in range(2):  # v chunk of emb_table rows
                for ec in range(2):  # e chunk
                    ptr0 = ips.tile([P, P], F32, tag="ptr0")
                    nc.tensor.transpose(
                        ptr0[:], etab_sb[:, c * E + ec * P : c * E + (ec + 1) * P], ident[:]
                    )
                    nc.vector.tensor_copy(
                        etT[:, ec * V + c * P : ec * V + (c + 1) * P], ptr0[:]
                    )
            for vc in range(2):  # embW row chunk
                for nh in range(2):  # embW col half
                    pe_ps = ips.tile([P, 512], F32, tag="pe_ps")
                    for ec in range(2):
                        nc.tensor.matmul(
                            pe_ps[:],
                            lhsT=etT[:, ec * V + vc * P : ec * V + (vc + 1) * P],
                            rhs=we_sb[:, ec * H + nh * 512 : ec * H + (nh + 1) * 512],
                            start=(ec == 0),
                            stop=(ec == 1),
                        )
                    ew_sb = isb.tile([P, 512], F32, tag="ew_sb")
                    nc.vector.tensor_copy(ew_sb[:], pe_ps[:])
                    nc.sync.dma_start(
                        embW_dram[vc * P : (vc + 1) * P, nh * 512 : (nh + 1) * 512],
                        ew_sb[:],
                    )

        # --- main pools
        gat = ctx.enter_context(tc.tile_pool(name="gat", bufs=6))
        work = ctx.enter_context(tc.tile_pool(name="work", bufs=4))
        psz = ctx.enter_context(tc.tile_pool(name="psz", bufs=4, space="PSUM"))
        pst = ctx.enter_context(tc.tile_pool(name="pst", bufs=2, space="PSUM"))
        psp = ctx.enter_context(tc.tile_pool(name="psp", bufs=2, space="PSUM"))

        def transpose_to_ring(h_ap, slot, nh):
            """h half [32, 512] (batch on partitions) -> ring chunks 4nh..4nh+3."""
            kh = KC // 2
            ptr = pst.tile([P, kh * BL], F32, tag="ptr", name=f"ptr{nh}")
            for j in range(kh):
                nc.tensor.transpose(
                    ptr[:, j * BL : (j + 1) * BL],
                    h_ap[:, j * P : (j + 1) * P],
                    ident[:BL, :BL],
                )
            # strided copy: ptr [128, (4, 32)] -> ring chunks (4nh+j) at slot
            dst = ring[:].rearrange("p (k sb) -> p k sb", k=KC)[
                :, nh * kh : (nh + 1) * kh, slot * BL : slot * BL + BL
            ]
            nc.vector.tensor_copy(dst, ptr[:].rearrange("p (k b) -> p k b", k=kh))

        # initial hidden -> ring slot (slots-1)
        h0_sb = work.tile([BL, H], F32, tag="h_sb")
        nc.sync.dma_start(h0_sb[:], h0_d[:])
        transpose_to_ring(h0_sb[:, :512], slots - 1, 0)
        transpose_to_ring(h0_sb[:, 512:], slots - 1, 1)

        prev_slot = slots - 1
        h_sb = None
        for t_rep in range(L_steps * repeat):
            t = t_rep % L_steps
            slot = t % slots
            # gather x_proj rows for this step
            xq = gat.tile([BL, H], F32, tag="xq")
            if ablate_gather:
                nc.sync.dma_start(xq[:], embW_dram[:BL, :])
            else:
                nc.gpsimd.indirect_dma_start(
                    out=xq[:],
                    out_offset=None,
                    in_=embW_dram[:, :],
                    in_offset=bass.IndirectOffsetOnAxis(ap=x_sb[:, t : t + 1], axis=0),
                )
            xqr = gat.tile([BL, H], F32R, tag="xqr")
            nc.vector.tensor_copy(xqr[:], xq[:])
            # z = ht_{t-1}.T @ W_h   (accumulated over KC chunks, 2 halves of 512)
            pz = []
            for nh in range(2):
                ps = psz.tile([BL, 512], F32, tag="psz")
                base = prev_slot * BL
                nc.tensor.matmul(
                    ps[:],
                    lhsT=identr[:],
                    rhs=xqr[:, nh * 512 : (nh + 1) * 512],
                    start=True,
                    stop=False,
                )
                for k in range(KC):
                    nc.tensor.matmul(
                        ps[:],
                        lhsT=ring[:, k * slots * BL + base : k * slots * BL + base + BL],
                        rhs=wh_sb[:, k * H + nh * 512 : k * H + nh * 512 + 512],
                        start=False,
                        stop=(k == KC - 1),
                    )
                pz.append(ps)
            # h = tanh(z + x_proj), then back to ring layout — per half so
            # chunks 0-3 are ready while half 1 is still in matmuls
            h_sb = work.tile([BL, H], F32, tag="h_sb")
            for nh in range(2):
                nc.scalar.activation(h_sb[:, nh * 512 : (nh + 1) * 512], pz[nh][:], TANH)
                transpose_to_ring(h_sb[:, nh * 512 : (nh + 1) * 512], slot, nh)
            prev_slot = slot

            # output projection over the last `blk` steps
            if t % blk == blk - 1:
                bi = t // blk
                par = (bi % 2) * blk * BL
                pps = []
                for mv in range(2):
                    pp = psp.tile([P, blk * BL], F32, tag="pp")
                    for k in range(KC):
                        nc.tensor.matmul(
                            pp[:],
                            lhsT=wo_sb[:, k * V + mv * P : k * V + (mv + 1) * P],
                            rhs=ring[:, k * slots * BL + par : k * slots * BL + par + blk * BL],
                            start=(k == 0),
                            stop=(k == KC - 1),
                        )
                    lg_sb = work.tile([P, blk * BL], F32, tag="lg", name=f"lg{mv}")
                    nc.vector.tensor_copy(lg_sb[:], pp[:])
                    pps.append(lg_sb)
                # transpose [v, b] tiles to [b, v] and assemble the block
                lgb = work.tile([BL, blk * V], F32, tag="lgb", bufs=2)
                for s in range(blk):
                    lgt = pst.tile([BL, V], F32, tag="ptr", name="lgt")
                    for mv in range(2):
                        nc.tensor.transpose(
                            lgt[:, mv * P : (mv + 1) * P],
                            pps[mv][:, s * BL : (s + 1) * BL],
                            ident[:],
                        )
                    nc.vector.tensor_copy(lgb[:, s * V : (s + 1) * V], lgt[:])
                nc.sync.dma_start(
                    lg_d[:, bi * blk : (bi + 1) * blk, :].rearrange("b s v -> b (s v)"),
                    lgb[:],
                )

        nc.sync.dma_start(hf_d[:], h_sb[:])
    nc.compile()
    return nc


_CACHE = {}


def _get_nc():
    if "nc" not in _CACHE:
        _CACHE["nc"] = build()
    return _CACHE["nc"]


def kernel(x, hidden, emb_table, W_e, W_h, W_o):
    x = np.ascontiguousarray(np.asarray(x).astype(np.int32))
    hidden = np.ascontiguousarray(np.asarray(hidden, dtype=np.float32))
    emb_table = np.ascontiguousarray(np.asarray(emb_table, dtype=np.float32))
    W_e = np.ascontiguousarray(np.asarray(W_e, dtype=np.float32))
    W_h = np.ascontiguousarray(np.asarray(W_h, dtype=np.float32))
    W_o = np.ascontiguousarray(np.asarray(W_o, dtype=np.float32))

    nc = _get_nc()
    in_maps = []
    for c in range(NCORES):
        sl = slice(c * BL, (c + 1) * BL)
        in_maps.append(
            {
                "x": x[sl],
                "hidden": hidden[sl],
                "emb_table": emb_table,
                "W_e": W_e,
                "W_h": W_h,
                "W_o": W_o,
            }
        )
    res = run_bass_kernel_spmd(nc, in_maps, core_ids=list(range(NCORES)), trace=False)
    logits = np.concatenate([res.results[c]["logits"] for c in range(NCORES)], axis=0)
    hfinal = np.concatenate([res.results[c]["hfinal"] for c in range(NCORES)], axis=0)
    return logits, hfinal
